# revision 3
# baseline (speedup 1.0000x reference)
"""Trainium2 Bass kernel for nn_CrossAttention (8-core data-parallel over batch).

Math (per batch b):
  x1 = x + PEx ; y1 = y + PEy           (raw-reshape positional encodings)
  q  = conv3x3(relu(conv3x3(x1,wq1)+bq1), wq2)+bq2   viewed as (1024,128)
  k  = conv3x3(relu(conv3x3(y1,wk1)+bk1), wk2)+bk2   viewed as (4096,128)
  out = softmax(s * q @ k.T) @ z.flat                (s = 1/sqrt(128))

Device mapping (one batch element per NeuronCore):
  - convs as 9 accumulating fp32r matmuls per output tile (weights stationary
    (ci,co), padded image moving with 2D shifted APs)
  - PE transposes conv outputs into j-major layout (t-major column order) for
    the attention contraction
  - logits in PSUM; softmax shift = stride-8 subsampled row max (any shift
    within ~[-80,+95] of the true max is exact after renormalization); online
    rescaling combines the 4 key-chunks
  - exp on ScalarE with fused per-partition bias + accumulated denominator;
    numerator via DVE scalar_tensor_tensor (P*v with fused sum)
"""

import numpy as np

import concourse.bass as bass
import concourse.mybir as mybir
import concourse.tile as tile
from concourse import bacc
from concourse.bass import ts
from concourse.bass_utils import run_bass_kernel_spmd

F32 = mybir.dt.float32
F32R = mybir.dt.float32r
BF16 = mybir.dt.bfloat16
AF = mybir.ActivationFunctionType
ALU = mybir.AluOpType

C = 128
A = 32          # q spatial side
H = 64          # k spatial side
SQ = A * A      # 1024
SK = H * H      # 4096
SCALE = float(C ** -0.5)
N_CORES = 8


def _make_pe(dim, length):
    pos = np.arange(length, dtype=np.float32)[:, None]
    div = np.exp(np.arange(0, dim, 2, dtype=np.float32) * np.float32(-np.log(10000.0) / dim))
    pe = np.zeros((length, dim), dtype=np.float32)
    pe[:, 0::2] = np.sin(pos * div)
    pe[:, 1::2] = np.cos(pos * div)
    return pe


def _build_program():
    nc = bacc.Bacc("TRN2", target_bir_lowering=False, debug=False, num_devices=N_CORES)

    dx = nc.dram_tensor("x", [C, SQ], F32, kind="ExternalInput")
    dy = nc.dram_tensor("y", [C, SK], F32, kind="ExternalInput")
    dv = nc.dram_tensor("vz", [1, SK], F32, kind="ExternalInput")
    dw = {n: nc.dram_tensor(n, [C, 9 * C], F32, kind="ExternalInput")
          for n in ("wq1", "wq2", "wk1", "wk2")}
    db = {n: nc.dram_tensor(n, [C, 1], F32, kind="ExternalInput")
          for n in ("bq1", "bq2", "bk1", "bk2")}
    dpex = nc.dram_tensor("pex", [C, SQ], F32, kind="ExternalInput")
    dpey = nc.dram_tensor("pey", [C, SK], F32, kind="ExternalInput")
    dident = nc.dram_tensor("ident", [C, C], F32, kind="ExternalInput")
    dout = nc.dram_tensor("out", [SQ, 1], F32, kind="ExternalOutput")

    XP, YP = A + 2, H + 2          # padded sides: 34, 66
    with tile.TileContext(nc) as tc:
        with (
            tc.tile_pool(name="const", bufs=1) as cst,
            tc.tile_pool(name="wstage", bufs=2) as wst,
            tc.tile_pool(name="kimg", bufs=2) as kip,
            tc.tile_pool(name="pp", bufs=3) as ppool,
            tc.tile_pool(name="scr", bufs=2) as scrp,
            tc.tile_pool(name="acc", bufs=2) as accp,
            tc.tile_pool(name="psc", bufs=2, space="PSUM") as psc,
            tc.tile_pool(name="pst", bufs=2, space="PSUM") as pst,
            tc.tile_pool(name="psa", bufs=2, space="PSUM") as psa,
        ):
            # ---- constants / inputs to SBUF ----
            w_r = {}
            for n in ("wq1", "wq2", "wk1", "wk2"):
                stg = wst.tile([C, 9 * C], F32, tag="wstg")
                nc.sync.dma_start(out=stg[:], in_=dw[n].ap())
                w_r[n] = cst.tile([C, 9 * C], F32R, tag=n, name=n + "_r")
                nc.vector.tensor_copy(w_r[n][:], stg[:])
            b_sb = {}
            for n in ("bq1", "bq2", "bk1", "bk2"):
                b_sb[n] = cst.tile([C, 1], F32, tag=n, name=n + "_sb")
                nc.sync.dma_start(out=b_sb[n][:], in_=db[n].ap())
            pex = cst.tile([C, SQ], F32, tag="pex")
            pey = cst.tile([C, SK], F32, tag="pey")
            ident = cst.tile([C, C], F32, tag="ident")
            nc.sync.dma_start(out=pex[:], in_=dpex.ap())
            nc.sync.dma_start(out=pey[:], in_=dpey.ap())
            nc.sync.dma_start(out=ident[:], in_=dident.ap())
            x_raw = cst.tile([C, SQ], F32, tag="x_raw")
            y_raw = cst.tile([C, SK], F32, tag="y_raw")
            nc.sync.dma_start(out=x_raw[:], in_=dx.ap())
            nc.sync.dma_start(out=y_raw[:], in_=dy.ap())
            v_rep = cst.tile([C, SK], F32, tag="v_rep")
            nc.sync.dma_start(out=v_rep[:], in_=dv.ap().broadcast_to((C, SK)))

            # ---- padded conv buffers (f32r) with zero borders ----
            zrow = cst.tile([C, YP], F32, tag="zrow")
            nc.vector.memset(zrow[:], 0.0)

            def pad_tile(tag, side):
                t = cst.tile([C, side * side], F32R, tag=tag, name=tag)
                t3 = t[:].rearrange("p (r c) -> p r c", c=side)
                zr = zrow[:, 0:side].rearrange("p (a c) -> p a c", a=1)
                zc = zrow[:, 0:side - 2].rearrange("p (r a) -> p r a", a=1)
                nc.vector.tensor_copy(t3[:, 0:1, :], zr)
                nc.vector.tensor_copy(t3[:, side - 1:side, :], zr)
                nc.vector.tensor_copy(t3[:, 1:side - 1, 0:1], zc)
                nc.vector.tensor_copy(t3[:, 1:side - 1, side - 1:side], zc)
                return t

            x_pad = pad_tile("x_pad", XP)
            y_pad = pad_tile("y_pad", YP)
            t1q = pad_tile("t1q", XP)
            t1k = pad_tile("t1k", YP)

            # x1 = x + PEx into padded interior (rounded to f32r)
            x_pad3 = x_pad[:].rearrange("p (r c) -> p r c", c=XP)
            nc.vector.tensor_tensor(
                out=x_pad3[:, 1:A + 1, 1:A + 1],
                in0=x_raw[:].rearrange("p (r c) -> p r c", c=A),
                in1=pex[:].rearrange("p (r c) -> p r c", c=A),
                op=ALU.add)
            y_pad3 = y_pad[:].rearrange("p (r c) -> p r c", c=YP)
            nc.vector.tensor_tensor(
                out=y_pad3[:, 1:H + 1, 1:H + 1],
                in0=y_raw[:].rearrange("p (r c) -> p r c", c=H),
                in1=pey[:].rearrange("p (r c) -> p r c", c=H),
                op=ALU.add)

            t1q3 = t1q[:].rearrange("p (r c) -> p r c", c=XP)
            t1k3 = t1k[:].rearrange("p (r c) -> p r c", c=YP)

            def conv_tile(src3, w, rows0, nrows, side_c):
                """9-tap accumulating fp32r matmuls -> psum (C, nrows*side_c)."""
                ps = psc.tile([C, nrows * side_c], F32, tag="cps")
                i = 0
                for dyy in range(3):
                    for dxx in range(3):
                        rhs = src3[:, rows0 + dyy: rows0 + dyy + nrows,
                                   dxx: dxx + side_c].bitcast(F32R)
                        nc.tensor.matmul(
                            ps[:].rearrange("p (r c) -> p r c", c=side_c),
                            w[:, ts(i, C)], rhs,
                            start=(i == 0), stop=(i == 8))
                        i += 1
                return ps

            # ---- q path ----
            q_img = cst.tile([C, SQ], F32, tag="q_img")
            qT = cst.tile([C, SQ], F32R, tag="qT")
            for n in range(2):   # tiles of 16 rows x 32 cols = 512
                ps1 = conv_tile(x_pad3, w_r["wq1"], 16 * n, 16, A)
                nc.scalar.activation(t1q3[:, 16 * n + 1:16 * n + 17, 1:A + 1],
                                     ps1[:].rearrange("p (r c) -> p r c", c=A),
                                     AF.Relu, bias=b_sb["bq1"][:])
            for n in range(2):
                ps2 = conv_tile(t1q3, w_r["wq2"], 16 * n, 16, A)
                nc.scalar.activation(q_img[:, ts(n, 512)], ps2[:],
                                     AF.Identity, bias=b_sb["bq2"][:])
            for g in range(2):   # transpose groups of 4 t-blocks
                pt = pst.tile([C, 512], F32, tag="tps")
                for i in range(4):
                    nc.tensor.transpose(pt[:, ts(i, C)],
                                        q_img[:, ts(4 * g + i, C)], ident[:])
                nc.scalar.activation(qT[:, ts(g, 512)], pt[:], AF.Copy)

            # ---- k path + attention, streamed in 4 chunks ----
            kT = cst.tile([C, SK], F32R, tag="kT")
            negM = cst.tile([C, 8], F32, tag="negM")
            denom = cst.tile([C, 8], F32, tag="denom")
            numer = cst.tile([C, 8], F32, tag="numer")

            def conv1_k(t):
                ps1 = conv_tile(y_pad3, w_r["wk1"], 8 * t, 8, H)
                nc.scalar.activation(t1k3[:, 8 * t + 1:8 * t + 9, 1:H + 1],
                                     ps1[:].rearrange("p (r c) -> p r c", c=H),
                                     AF.Relu, bias=b_sb["bk1"][:])

            def conv2_k(t):
                ps2 = conv_tile(t1k3, w_r["wk2"], 8 * t, 8, H)
                kimg = kip.tile([C, 512], F32, tag="kimg")
                nc.scalar.activation(kimg[:], ps2[:], AF.Identity, bias=b_sb["bk2"][:])
                pt = pst.tile([C, 512], F32, tag="tps")
                for i in range(4):
                    nc.tensor.transpose(pt[:, ts(i, C)], kimg[:, ts(i, C)], ident[:])
                nc.scalar.activation(kT[:, ts(t, 512)], pt[:], AF.Copy)

            conv1_k(0)
            conv1_k(1)
            for c in range(4):
                if 2 * c + 2 < 8:
                    conv1_k(2 * c + 2)
                if 2 * c + 3 < 8:
                    conv1_k(2 * c + 3)
                conv2_k(2 * c)
                conv2_k(2 * c + 1)

                negmax_c = accp.tile([C, 8], F32, tag="negmax_c")
                d_c = accp.tile([C, 8], F32, tag="d_c")
                n_c = accp.tile([C, 8], F32, tag="n_c")
                for m in range(8):
                    psl = psa.tile([C, 1024], F32, tag="psl")
                    for u in range(2):
                        nc.tensor.matmul(psl[:, ts(u, 512)], qT[:, ts(m, C)],
                                         kT[:, 1024 * c + 512 * u: 1024 * c + 512 * (u + 1)],
                                         start=True, stop=True)
                    # shift: minus the max over one t-block (every-8th key)
                    nc.vector.tensor_reduce(out=negmax_c[:, m:m + 1], in_=psl[:, 0:C],
                                            axis=mybir.AxisListType.X, op=ALU.max,
                                            negate=True)
                    bias_m = scrp.tile([C, 1], F32, tag="bias_m")
                    nc.vector.tensor_scalar(out=bias_m[:], in0=negmax_c[:, m:m + 1],
                                            scalar1=SCALE, scalar2=None, op0=ALU.mult)
                    P = ppool.tile([C, 1024], F32, tag="P")
                    nc.scalar.activation(P[:], psl[:], AF.Exp, bias=bias_m[:],
                                         scale=SCALE, accum_out=d_c[:, m:m + 1])
                    scrap = scrp.tile([C, 1024], BF16, tag="scrap")
                    nc.vector.scalar_tensor_tensor(out=scrap[:], in0=P[:], scalar=1.0,
                                                   in1=v_rep[:, ts(c, 1024)],
                                                   op0=ALU.bypass, op1=ALU.mult,
                                                   accum_out=n_c[:, m:m + 1])
                if c == 0:
                    nc.vector.tensor_copy(negM[:], negmax_c[:])
                    nc.vector.tensor_copy(denom[:], d_c[:])
                    nc.vector.tensor_copy(numer[:], n_c[:])
                else:
                    nmin = scrp.tile([C, 8], F32, tag="nmin")
                    diffs = scrp.tile([C, 16], F32, tag="diffs")
                    alphas = scrp.tile([C, 16], F32, tag="alphas")
                    t8 = scrp.tile([C, 8], F32, tag="t8")
                    nc.vector.tensor_tensor(out=nmin[:], in0=negM[:], in1=negmax_c[:], op=ALU.min)
                    nc.vector.tensor_tensor(out=diffs[:, 0:8], in0=nmin[:], in1=negM[:], op=ALU.subtract)
                    nc.vector.tensor_tensor(out=diffs[:, 8:16], in0=nmin[:], in1=negmax_c[:], op=ALU.subtract)
                    nc.scalar.activation(alphas[:], diffs[:], AF.Exp, scale=SCALE)
                    nc.vector.tensor_tensor(out=denom[:], in0=denom[:], in1=alphas[:, 0:8], op=ALU.mult)
                    nc.vector.tensor_tensor(out=t8[:], in0=d_c[:], in1=alphas[:, 8:16], op=ALU.mult)
                    nc.vector.tensor_tensor(out=denom[:], in0=denom[:], in1=t8[:], op=ALU.add)
                    nc.vector.tensor_tensor(out=numer[:], in0=numer[:], in1=alphas[:, 0:8], op=ALU.mult)
                    nc.vector.tensor_tensor(out=t8[:], in0=n_c[:], in1=alphas[:, 8:16], op=ALU.mult)
                    nc.vector.tensor_tensor(out=numer[:], in0=numer[:], in1=t8[:], op=ALU.add)
                    nc.vector.tensor_copy(negM[:], nmin[:])

            recip = cst.tile([C, 8], F32, tag="recip")
            res = cst.tile([C, 8], F32, tag="res")
            nc.vector.reciprocal(recip[:], denom[:])
            nc.vector.tensor_tensor(out=res[:], in0=numer[:], in1=recip[:], op=ALU.mult)
            nc.sync.dma_start(out=dout.ap().rearrange("(co m) one -> co (m one)", m=8),
                              in_=res[:])

    nc.compile()
    return nc


_NC_CACHE = []


def kernel(x, y, z, wq1, bq1, wq2, bq2, wk1, bk1, wk2, bk2):
    x = np.asarray(x, dtype=np.float32)
    y = np.asarray(y, dtype=np.float32)
    z = np.asarray(z, dtype=np.float32)
    B = x.shape[0]
    assert B == N_CORES

    if not _NC_CACHE:
        _NC_CACHE.append(_build_program())
    nc = _NC_CACHE[0]

    # weights: (co, ci, dy, dx) -> (ci, tap*128+co)
    wmap = {}
    for name, w in (("wq1", wq1), ("wq2", wq2), ("wk1", wk1), ("wk2", wk2)):
        wmap[name] = np.ascontiguousarray(
            np.asarray(w, dtype=np.float32).transpose(1, 2, 3, 0).reshape(C, 9 * C))
    bmap = {"bq1": bq1, "bq2": bq2, "bk1": bk1, "bk2": bk2}
    bmap = {n: np.ascontiguousarray(np.asarray(b, dtype=np.float32).reshape(C, 1))
            for n, b in bmap.items()}
    pex = np.ascontiguousarray(_make_pe(C, SQ).reshape(C, SQ))
    pey = np.ascontiguousarray(_make_pe(C, SK).reshape(C, SK))
    ident = np.eye(C, dtype=np.float32)
    # v in t-major key order: store[t*128+co] = z_flat[co*32+t]
    zperm = np.ascontiguousarray(
        z.reshape(B, SK).reshape(B, C, SK // C).transpose(0, 2, 1).reshape(B, 1, SK))

    in_maps = []
    for b in range(B):
        m = {
            "x": np.ascontiguousarray(x[b].reshape(C, SQ)),
            "y": np.ascontiguousarray(y[b].reshape(C, SK)),
            "vz": zperm[b],
            "pex": pex, "pey": pey, "ident": ident,
        }
        m.update(wmap)
        m.update(bmap)
        in_maps.append(m)

    res = run_bass_kernel_spmd(nc, in_maps, core_ids=list(range(N_CORES)))
    out = np.stack([res.results[b]["out"].reshape(SQ, 1) for b in range(B)])
    return out.astype(np.float32)


# revision 4
# speedup vs baseline: 36.7942x; 36.7942x over previous
"""Trainium2 Bass kernel for nn_CrossAttention (8-core data-parallel over batch).

Math (per batch b):
  x1 = x + PEx ; y1 = y + PEy           (raw-reshape positional encodings)
  q  = conv3x3(relu(conv3x3(x1,wq1)+bq1), wq2)+bq2   viewed as (1024,128)
  k  = conv3x3(relu(conv3x3(y1,wk1)+bk1), wk2)+bk2   viewed as (4096,128)
  out = softmax(s * q @ k.T) @ z.flat                (s = 1/sqrt(128))

Device mapping (one batch element per NeuronCore):
  - convs as 9 accumulating fp32r matmuls per output tile (weights stationary
    (ci,co), padded image moving with 2D shifted APs)
  - PE transposes conv outputs into j-major layout (t-major column order) for
    the attention contraction
  - logits in PSUM; softmax shift = stride-8 subsampled row max (any shift
    within ~[-80,+95] of the true max is exact after renormalization); online
    rescaling combines the 4 key-chunks
  - exp on ScalarE with fused per-partition bias + accumulated denominator;
    numerator via DVE scalar_tensor_tensor (P*v with fused sum)
"""

import numpy as np

import concourse.bass as bass
import concourse.mybir as mybir
import concourse.tile as tile
from concourse import bacc
from concourse.bass import ts
from concourse.bass_utils import run_bass_kernel_spmd

F32 = mybir.dt.float32
F32R = mybir.dt.float32r
BF16 = mybir.dt.bfloat16
AF = mybir.ActivationFunctionType
ALU = mybir.AluOpType

C = 128
A = 32          # q spatial side
H = 64          # k spatial side
SQ = A * A      # 1024
SK = H * H      # 4096
SCALE = float(C ** -0.5)
N_CORES = 8


def _make_pe(dim, length):
    pos = np.arange(length, dtype=np.float32)[:, None]
    div = np.exp(np.arange(0, dim, 2, dtype=np.float32) * np.float32(-np.log(10000.0) / dim))
    pe = np.zeros((length, dim), dtype=np.float32)
    pe[:, 0::2] = np.sin(pos * div)
    pe[:, 1::2] = np.cos(pos * div)
    return pe


def _build_program(repeat=1):
    nc = bacc.Bacc("TRN2", target_bir_lowering=False, debug=False, num_devices=N_CORES)

    dx = nc.dram_tensor("x", [C, SQ], F32, kind="ExternalInput")
    dy = nc.dram_tensor("y", [C, SK], F32, kind="ExternalInput")
    dv = nc.dram_tensor("vz", [1, SK], F32, kind="ExternalInput")
    dw = {n: nc.dram_tensor(n, [C, 9 * C], F32, kind="ExternalInput")
          for n in ("wq1", "wq2", "wk1", "wk2")}
    db = {n: nc.dram_tensor(n, [C, 1], F32, kind="ExternalInput")
          for n in ("bq1", "bq2", "bk1", "bk2")}
    dpex = nc.dram_tensor("pex", [C, SQ], F32, kind="ExternalInput")
    dpey = nc.dram_tensor("pey", [C, SK], F32, kind="ExternalInput")
    dident = nc.dram_tensor("ident", [C, C], F32, kind="ExternalInput")
    dout = nc.dram_tensor("out", [SQ, 1], F32, kind="ExternalOutput")

    XP, YP = A + 2, H + 2          # padded sides: 34, 66
    with tile.TileContext(nc) as tc:
        with (
            tc.tile_pool(name="const", bufs=1) as cst,
            tc.tile_pool(name="wstage", bufs=2) as wst,
            tc.tile_pool(name="kimg", bufs=2) as kip,
            tc.tile_pool(name="pp", bufs=3) as ppool,
            tc.tile_pool(name="scr", bufs=2) as scrp,
            tc.tile_pool(name="acc", bufs=2) as accp,
            tc.tile_pool(name="psc", bufs=2, space="PSUM") as psc,
            tc.tile_pool(name="pst", bufs=2, space="PSUM") as pst,
            tc.tile_pool(name="psa", bufs=2, space="PSUM") as psa,
        ):
          import contextlib
          loop_cm = (tc.For_i(0, repeat, 1,
                              hint_engines=(mybir.EngineType.PE, mybir.EngineType.Activation,
                                            mybir.EngineType.DVE, mybir.EngineType.SP))
                     if repeat > 1 else contextlib.nullcontext())
          with loop_cm:
            # ---- constants / inputs to SBUF ----
            w_r = {}
            for n in ("wq1", "wq2", "wk1", "wk2"):
                stg = wst.tile([C, 9 * C], F32, tag="wstg")
                nc.sync.dma_start(out=stg[:], in_=dw[n].ap())
                w_r[n] = cst.tile([C, 9 * C], F32R, tag=n, name=n + "_r")
                nc.vector.tensor_copy(w_r[n][:], stg[:])
            b_sb = {}
            for n in ("bq1", "bq2", "bk1", "bk2"):
                b_sb[n] = cst.tile([C, 1], F32, tag=n, name=n + "_sb")
                nc.sync.dma_start(out=b_sb[n][:], in_=db[n].ap())
            pex = cst.tile([C, SQ], F32, tag="pex")
            pey = cst.tile([C, SK], F32, tag="pey")
            ident = cst.tile([C, C], F32, tag="ident")
            nc.sync.dma_start(out=pex[:], in_=dpex.ap())
            nc.sync.dma_start(out=pey[:], in_=dpey.ap())
            nc.sync.dma_start(out=ident[:], in_=dident.ap())
            x_raw = cst.tile([C, SQ], F32, tag="x_raw")
            y_raw = cst.tile([C, SK], F32, tag="y_raw")
            nc.sync.dma_start(out=x_raw[:], in_=dx.ap())
            nc.sync.dma_start(out=y_raw[:], in_=dy.ap())
            v_rep = cst.tile([C, SK], F32, tag="v_rep")
            nc.sync.dma_start(out=v_rep[:], in_=dv.ap().broadcast_to((C, SK)))

            # ---- padded conv buffers (f32r) with zero borders ----
            zrow = cst.tile([C, YP], F32, tag="zrow")
            nc.vector.memset(zrow[:], 0.0)

            def pad_tile(tag, side):
                t = cst.tile([C, side * side], F32R, tag=tag, name=tag)
                t3 = t[:].rearrange("p (r c) -> p r c", c=side)
                zr = zrow[:, 0:side].rearrange("p (a c) -> p a c", a=1)
                zc = zrow[:, 0:side - 2].rearrange("p (r a) -> p r a", a=1)
                nc.vector.tensor_copy(t3[:, 0:1, :], zr)
                nc.vector.tensor_copy(t3[:, side - 1:side, :], zr)
                nc.vector.tensor_copy(t3[:, 1:side - 1, 0:1], zc)
                nc.vector.tensor_copy(t3[:, 1:side - 1, side - 1:side], zc)
                return t

            x_pad = pad_tile("x_pad", XP)
            y_pad = pad_tile("y_pad", YP)
            t1q = pad_tile("t1q", XP)
            t1k = pad_tile("t1k", YP)

            # x1 = x + PEx into padded interior (rounded to f32r)
            x_pad3 = x_pad[:].rearrange("p (r c) -> p r c", c=XP)
            nc.vector.tensor_tensor(
                out=x_pad3[:, 1:A + 1, 1:A + 1],
                in0=x_raw[:].rearrange("p (r c) -> p r c", c=A),
                in1=pex[:].rearrange("p (r c) -> p r c", c=A),
                op=ALU.add)
            y_pad3 = y_pad[:].rearrange("p (r c) -> p r c", c=YP)
            nc.vector.tensor_tensor(
                out=y_pad3[:, 1:H + 1, 1:H + 1],
                in0=y_raw[:].rearrange("p (r c) -> p r c", c=H),
                in1=pey[:].rearrange("p (r c) -> p r c", c=H),
                op=ALU.add)

            t1q3 = t1q[:].rearrange("p (r c) -> p r c", c=XP)
            t1k3 = t1k[:].rearrange("p (r c) -> p r c", c=YP)

            def conv_tile(src3, w, rows0, nrows, side_c):
                """9-tap accumulating fp32r matmuls -> psum (C, nrows*side_c)."""
                ps = psc.tile([C, nrows * side_c], F32, tag="cps")
                i = 0
                for dyy in range(3):
                    for dxx in range(3):
                        rhs = src3[:, rows0 + dyy: rows0 + dyy + nrows,
                                   dxx: dxx + side_c].bitcast(F32R)
                        nc.tensor.matmul(
                            ps[:].rearrange("p (r c) -> p r c", c=side_c),
                            w[:, ts(i, C)], rhs,
                            start=(i == 0), stop=(i == 8))
                        i += 1
                return ps

            # ---- q path ----
            q_img = cst.tile([C, SQ], F32, tag="q_img")
            qT = cst.tile([C, SQ], F32R, tag="qT")
            for n in range(2):   # tiles of 16 rows x 32 cols = 512
                ps1 = conv_tile(x_pad3, w_r["wq1"], 16 * n, 16, A)
                nc.scalar.activation(t1q3[:, 16 * n + 1:16 * n + 17, 1:A + 1],
                                     ps1[:].rearrange("p (r c) -> p r c", c=A),
                                     AF.Relu, bias=b_sb["bq1"][:])
            for n in range(2):
                ps2 = conv_tile(t1q3, w_r["wq2"], 16 * n, 16, A)
                nc.scalar.activation(q_img[:, ts(n, 512)], ps2[:],
                                     AF.Identity, bias=b_sb["bq2"][:])
            for g in range(2):   # transpose groups of 4 t-blocks
                pt = pst.tile([C, 512], F32, tag="tps")
                for i in range(4):
                    nc.tensor.transpose(pt[:, ts(i, C)],
                                        q_img[:, ts(4 * g + i, C)], ident[:])
                nc.scalar.activation(qT[:, ts(g, 512)], pt[:], AF.Copy)

            # ---- k path + attention, streamed in 4 chunks ----
            kT = cst.tile([C, SK], F32R, tag="kT")
            negM = cst.tile([C, 8], F32, tag="negM")
            denom = cst.tile([C, 8], F32, tag="denom")
            numer = cst.tile([C, 8], F32, tag="numer")

            def conv1_k(t):
                ps1 = conv_tile(y_pad3, w_r["wk1"], 8 * t, 8, H)
                nc.scalar.activation(t1k3[:, 8 * t + 1:8 * t + 9, 1:H + 1],
                                     ps1[:].rearrange("p (r c) -> p r c", c=H),
                                     AF.Relu, bias=b_sb["bk1"][:])

            def conv2_k(t):
                ps2 = conv_tile(t1k3, w_r["wk2"], 8 * t, 8, H)
                kimg = kip.tile([C, 512], F32, tag="kimg")
                nc.scalar.activation(kimg[:], ps2[:], AF.Identity, bias=b_sb["bk2"][:])
                pt = pst.tile([C, 512], F32, tag="tps")
                for i in range(4):
                    nc.tensor.transpose(pt[:, ts(i, C)], kimg[:, ts(i, C)], ident[:])
                nc.scalar.activation(kT[:, ts(t, 512)], pt[:], AF.Copy)

            conv1_k(0)
            conv1_k(1)
            for c in range(4):
                if 2 * c + 2 < 8:
                    conv1_k(2 * c + 2)
                if 2 * c + 3 < 8:
                    conv1_k(2 * c + 3)
                conv2_k(2 * c)
                conv2_k(2 * c + 1)

                negmax_c = accp.tile([C, 8], F32, tag="negmax_c")
                d_c = accp.tile([C, 8], F32, tag="d_c")
                n_c = accp.tile([C, 8], F32, tag="n_c")
                for m in range(8):
                    psl = psa.tile([C, 1024], F32, tag="psl")
                    for u in range(2):
                        nc.tensor.matmul(psl[:, ts(u, 512)], qT[:, ts(m, C)],
                                         kT[:, 1024 * c + 512 * u: 1024 * c + 512 * (u + 1)],
                                         start=True, stop=True)
                    # shift: minus the max over one t-block (every-8th key)
                    nc.vector.tensor_reduce(out=negmax_c[:, m:m + 1], in_=psl[:, 0:C],
                                            axis=mybir.AxisListType.X, op=ALU.max,
                                            negate=True)
                    bias_m = scrp.tile([C, 1], F32, tag="bias_m")
                    nc.vector.tensor_scalar(out=bias_m[:], in0=negmax_c[:, m:m + 1],
                                            scalar1=SCALE, scalar2=None, op0=ALU.mult)
                    P = ppool.tile([C, 1024], F32, tag="P")
                    nc.scalar.activation(P[:], psl[:], AF.Exp, bias=bias_m[:],
                                         scale=SCALE, accum_out=d_c[:, m:m + 1])
                    scrap = scrp.tile([C, 1024], BF16, tag="scrap")
                    nc.vector.scalar_tensor_tensor(out=scrap[:], in0=P[:], scalar=1.0,
                                                   in1=v_rep[:, ts(c, 1024)],
                                                   op0=ALU.bypass, op1=ALU.mult,
                                                   accum_out=n_c[:, m:m + 1])
                if c == 0:
                    nc.vector.tensor_copy(negM[:], negmax_c[:])
                    nc.vector.tensor_copy(denom[:], d_c[:])
                    nc.vector.tensor_copy(numer[:], n_c[:])
                else:
                    nmin = scrp.tile([C, 8], F32, tag="nmin")
                    diffs = scrp.tile([C, 16], F32, tag="diffs")
                    alphas = scrp.tile([C, 16], F32, tag="alphas")
                    t8 = scrp.tile([C, 8], F32, tag="t8")
                    nc.vector.tensor_tensor(out=nmin[:], in0=negM[:], in1=negmax_c[:], op=ALU.min)
                    nc.vector.tensor_tensor(out=diffs[:, 0:8], in0=nmin[:], in1=negM[:], op=ALU.subtract)
                    nc.vector.tensor_tensor(out=diffs[:, 8:16], in0=nmin[:], in1=negmax_c[:], op=ALU.subtract)
                    nc.scalar.activation(alphas[:], diffs[:], AF.Exp, scale=SCALE)
                    nc.vector.tensor_tensor(out=denom[:], in0=denom[:], in1=alphas[:, 0:8], op=ALU.mult)
                    nc.vector.tensor_tensor(out=t8[:], in0=d_c[:], in1=alphas[:, 8:16], op=ALU.mult)
                    nc.vector.tensor_tensor(out=denom[:], in0=denom[:], in1=t8[:], op=ALU.add)
                    nc.vector.tensor_tensor(out=numer[:], in0=numer[:], in1=alphas[:, 0:8], op=ALU.mult)
                    nc.vector.tensor_tensor(out=t8[:], in0=n_c[:], in1=alphas[:, 8:16], op=ALU.mult)
                    nc.vector.tensor_tensor(out=numer[:], in0=numer[:], in1=t8[:], op=ALU.add)
                    nc.vector.tensor_copy(negM[:], nmin[:])

            recip = cst.tile([C, 8], F32, tag="recip")
            res = cst.tile([C, 8], F32, tag="res")
            nc.vector.reciprocal(recip[:], denom[:])
            nc.vector.tensor_tensor(out=res[:], in0=numer[:], in1=recip[:], op=ALU.mult)
            nc.sync.dma_start(out=dout.ap().rearrange("(co m) one -> co (m one)", m=8),
                              in_=res[:])

    nc.compile()
    return nc


_NC_CACHE = []


def kernel(x, y, z, wq1, bq1, wq2, bq2, wk1, bk1, wk2, bk2):
    x = np.asarray(x, dtype=np.float32)
    y = np.asarray(y, dtype=np.float32)
    z = np.asarray(z, dtype=np.float32)
    B = x.shape[0]
    assert B == N_CORES

    if not _NC_CACHE:
        _NC_CACHE.append(_build_program())
    nc = _NC_CACHE[0]

    # weights: (co, ci, dy, dx) -> (ci, tap*128+co)
    wmap = {}
    for name, w in (("wq1", wq1), ("wq2", wq2), ("wk1", wk1), ("wk2", wk2)):
        wmap[name] = np.ascontiguousarray(
            np.asarray(w, dtype=np.float32).transpose(1, 2, 3, 0).reshape(C, 9 * C))
    bmap = {"bq1": bq1, "bq2": bq2, "bk1": bk1, "bk2": bk2}
    bmap = {n: np.ascontiguousarray(np.asarray(b, dtype=np.float32).reshape(C, 1))
            for n, b in bmap.items()}
    pex = np.ascontiguousarray(_make_pe(C, SQ).reshape(C, SQ))
    pey = np.ascontiguousarray(_make_pe(C, SK).reshape(C, SK))
    ident = np.eye(C, dtype=np.float32)
    # v in t-major key order: store[t*128+co] = z_flat[co*32+t]
    zperm = np.ascontiguousarray(
        z.reshape(B, SK).reshape(B, C, SK // C).transpose(0, 2, 1).reshape(B, 1, SK))

    in_maps = []
    for b in range(B):
        m = {
            "x": np.ascontiguousarray(x[b].reshape(C, SQ)),
            "y": np.ascontiguousarray(y[b].reshape(C, SK)),
            "vz": zperm[b],
            "pex": pex, "pey": pey, "ident": ident,
        }
        m.update(wmap)
        m.update(bmap)
        in_maps.append(m)

    res = run_bass_kernel_spmd(nc, in_maps, core_ids=list(range(N_CORES)))
    out = np.stack([res.results[b]["out"].reshape(SQ, 1) for b in range(B)])
    return out.astype(np.float32)


# revision 5
# speedup vs baseline: 72.4827x; 1.9699x over previous
"""Trainium2 Bass kernel for nn_CrossAttention (8-core data-parallel over batch).

Math (per batch b):
  x1 = x + PEx ; y1 = y + PEy           (raw-reshape positional encodings)
  q  = conv3x3(relu(conv3x3(x1,wq1)+bq1), wq2)+bq2   viewed as (1024,128)
  k  = conv3x3(relu(conv3x3(y1,wk1)+bk1), wk2)+bk2   viewed as (4096,128)
  out = softmax(s * q @ k.T) @ z.flat                (s = 1/sqrt(128))

Device mapping (one batch element per NeuronCore):
  - convs as 9 accumulating fp32r matmuls per output tile (weights stationary
    (ci,co), padded image moving with 2D shifted APs)
  - PE transposes conv outputs into j-major layout (t-major column order) for
    the attention contraction
  - logits in PSUM; softmax shift = stride-8 subsampled row max (any shift
    within ~[-80,+95] of the true max is exact after renormalization); online
    rescaling combines the 4 key-chunks
  - exp on ScalarE with fused per-partition bias + accumulated denominator;
    numerator via DVE scalar_tensor_tensor (P*v with fused sum)
"""

import numpy as np

import concourse.bass as bass
import concourse.mybir as mybir
import concourse.tile as tile
from concourse import bacc
from concourse.bass import ts
from concourse.bass_utils import run_bass_kernel_spmd

F32 = mybir.dt.float32
F32R = mybir.dt.float32r
BF16 = mybir.dt.bfloat16
AF = mybir.ActivationFunctionType
ALU = mybir.AluOpType

C = 128
A = 32          # q spatial side
H = 64          # k spatial side
SQ = A * A      # 1024
SK = H * H      # 4096
SCALE = float(C ** -0.5)
N_CORES = 8


def _make_pe(dim, length):
    pos = np.arange(length, dtype=np.float32)[:, None]
    div = np.exp(np.arange(0, dim, 2, dtype=np.float32) * np.float32(-np.log(10000.0) / dim))
    pe = np.zeros((length, dim), dtype=np.float32)
    pe[:, 0::2] = np.sin(pos * div)
    pe[:, 1::2] = np.cos(pos * div)
    return pe


def _build_program(repeat=1):
    nc = bacc.Bacc("TRN2", target_bir_lowering=False, debug=False, num_devices=N_CORES)

    dx = nc.dram_tensor("x", [C, SQ], F32, kind="ExternalInput")
    dy = nc.dram_tensor("y", [C, SK], F32, kind="ExternalInput")
    dv = nc.dram_tensor("vz", [1, SK], F32, kind="ExternalInput")
    dw = {n: nc.dram_tensor(n, [C, 9 * C], F32, kind="ExternalInput")
          for n in ("wq1", "wq2", "wk1", "wk2")}
    db = {n: nc.dram_tensor(n, [C, 1], F32, kind="ExternalInput")
          for n in ("bq1", "bq2", "bk1", "bk2")}
    dpex = nc.dram_tensor("pex", [C, SQ], F32, kind="ExternalInput")
    dpey = nc.dram_tensor("pey", [C, SK], F32, kind="ExternalInput")
    dident = nc.dram_tensor("ident", [C, C], F32, kind="ExternalInput")
    dout = nc.dram_tensor("out", [SQ, 1], F32, kind="ExternalOutput")

    XP, YP = A + 2, H + 2          # padded sides: 34, 66
    with tile.TileContext(nc) as tc:
        with (
            tc.tile_pool(name="const", bufs=1) as cst,
            tc.tile_pool(name="wstage", bufs=2) as wst,
            tc.tile_pool(name="kimg", bufs=2) as kip,
            tc.tile_pool(name="pp", bufs=3) as ppool,
            tc.tile_pool(name="scr", bufs=2) as scrp,
            tc.tile_pool(name="acc", bufs=2) as accp,
            tc.tile_pool(name="psc", bufs=2, space="PSUM") as psc,
            tc.tile_pool(name="pst", bufs=2, space="PSUM") as pst,
            tc.tile_pool(name="psa", bufs=2, space="PSUM") as psa,
        ):
          import contextlib
          loop_cm = (tc.For_i(0, repeat, 1,
                              hint_engines=(mybir.EngineType.PE, mybir.EngineType.Activation,
                                            mybir.EngineType.DVE, mybir.EngineType.SP))
                     if repeat > 1 else contextlib.nullcontext())
          with loop_cm:
            # ---- constants / inputs to SBUF (q-critical first) ----
            w_r, b_sb = {}, {}
            x_raw = cst.tile([C, SQ], F32, tag="x_raw")
            pex = cst.tile([C, SQ], F32, tag="pex")
            nc.sync.dma_start(out=x_raw[:], in_=dx.ap())
            nc.sync.dma_start(out=pex[:], in_=dpex.ap())

            def load_w(n):
                stg = wst.tile([C, 9 * C], F32, tag="wstg", name="stg_" + n)
                nc.sync.dma_start(out=stg[:, 0:576], in_=dw[n].ap()[:, 0:576])
                nc.sync.dma_start(out=stg[:, 576:1152], in_=dw[n].ap()[:, 576:1152])
                w_r[n] = cst.tile([C, 9 * C], F32R, tag=n, name=n + "_r")
                nc.vector.tensor_copy(w_r[n][:], stg[:])

            def load_b(n):
                b_sb[n] = cst.tile([C, 1], F32, tag=n, name=n + "_sb")
                nc.sync.dma_start(out=b_sb[n][:], in_=db[n].ap())

            load_w("wq1"); load_b("bq1"); load_w("wq2"); load_b("bq2")
            y_raw = cst.tile([C, SK], F32, tag="y_raw")
            pey = cst.tile([C, SK], F32, tag="pey")
            for h in range(2):
                nc.sync.dma_start(out=y_raw[:, ts(h, SK // 2)], in_=dy.ap()[:, ts(h, SK // 2)])
                nc.sync.dma_start(out=pey[:, ts(h, SK // 2)], in_=dpey.ap()[:, ts(h, SK // 2)])
            load_w("wk1"); load_b("bk1"); load_w("wk2"); load_b("bk2")
            ident = cst.tile([C, C], F32, tag="ident")
            nc.sync.dma_start(out=ident[:], in_=dident.ap())
            v_rep = cst.tile([C, SK], F32, tag="v_rep")
            for h in range(2):
                nc.sync.dma_start(out=v_rep[:, ts(h, SK // 2)],
                                  in_=dv.ap()[:, ts(h, SK // 2)].broadcast_to((C, SK // 2)))

            # ---- padded conv buffers (f32r) with zero borders ----
            zrow = cst.tile([C, YP], F32, tag="zrow")
            nc.vector.memset(zrow[:], 0.0)

            def pad_tile(tag, side):
                t = cst.tile([C, side * side], F32R, tag=tag, name=tag)
                t3 = t[:].rearrange("p (r c) -> p r c", c=side)
                zr = zrow[:, 0:side].rearrange("p (a c) -> p a c", a=1)
                zc = zrow[:, 0:side - 2].rearrange("p (r a) -> p r a", a=1)
                nc.vector.tensor_copy(t3[:, 0:1, :], zr)
                nc.vector.tensor_copy(t3[:, side - 1:side, :], zr)
                nc.vector.tensor_copy(t3[:, 1:side - 1, 0:1], zc)
                nc.vector.tensor_copy(t3[:, 1:side - 1, side - 1:side], zc)
                return t

            x_pad = pad_tile("x_pad", XP)
            y_pad = pad_tile("y_pad", YP)
            t1q = pad_tile("t1q", XP)
            t1k = pad_tile("t1k", YP)

            # x1 = x + PEx into padded interior (rounded to f32r)
            x_pad3 = x_pad[:].rearrange("p (r c) -> p r c", c=XP)
            nc.vector.tensor_tensor(
                out=x_pad3[:, 1:A + 1, 1:A + 1],
                in0=x_raw[:].rearrange("p (r c) -> p r c", c=A),
                in1=pex[:].rearrange("p (r c) -> p r c", c=A),
                op=ALU.add)
            y_pad3 = y_pad[:].rearrange("p (r c) -> p r c", c=YP)
            nc.vector.tensor_tensor(
                out=y_pad3[:, 1:H + 1, 1:H + 1],
                in0=y_raw[:].rearrange("p (r c) -> p r c", c=H),
                in1=pey[:].rearrange("p (r c) -> p r c", c=H),
                op=ALU.add)

            t1q3 = t1q[:].rearrange("p (r c) -> p r c", c=XP)
            t1k3 = t1k[:].rearrange("p (r c) -> p r c", c=YP)

            def conv_tile(src3, w, rows0, nrows, side_c):
                """9-tap accumulating fp32r matmuls -> psum (C, nrows*side_c)."""
                ps = psc.tile([C, nrows * side_c], F32, tag="cps")
                i = 0
                for dyy in range(3):
                    for dxx in range(3):
                        rhs = src3[:, rows0 + dyy: rows0 + dyy + nrows,
                                   dxx: dxx + side_c].bitcast(F32R)
                        nc.tensor.matmul(
                            ps[:].rearrange("p (r c) -> p r c", c=side_c),
                            w[:, ts(i, C)], rhs,
                            start=(i == 0), stop=(i == 8))
                        i += 1
                return ps

            # ---- q path ----
            q_img = cst.tile([C, SQ], F32, tag="q_img")
            qT = cst.tile([C, SQ], F32R, tag="qT")
            for n in range(2):   # tiles of 16 rows x 32 cols = 512
                ps1 = conv_tile(x_pad3, w_r["wq1"], 16 * n, 16, A)
                nc.scalar.activation(t1q3[:, 16 * n + 1:16 * n + 17, 1:A + 1],
                                     ps1[:].rearrange("p (r c) -> p r c", c=A),
                                     AF.Relu, bias=b_sb["bq1"][:])
            for n in range(2):
                ps2 = conv_tile(t1q3, w_r["wq2"], 16 * n, 16, A)
                nc.scalar.activation(q_img[:, ts(n, 512)], ps2[:],
                                     AF.Identity, bias=b_sb["bq2"][:])
            for g in range(2):   # transpose groups of 4 t-blocks; qT pre-scaled by s
                pt = pst.tile([C, 512], F32, tag="tps")
                for i in range(4):
                    nc.tensor.transpose(pt[:, ts(i, C)],
                                        q_img[:, ts(4 * g + i, C)], ident[:])
                nc.vector.tensor_scalar(out=qT[:, ts(g, 512)], in0=pt[:],
                                        scalar1=SCALE, scalar2=None, op0=ALU.mult)

            # ---- k path + attention, streamed in 4 chunks ----
            kT = cst.tile([C, SK], F32R, tag="kT")
            negM = cst.tile([C, 8], F32, tag="negM")
            denom = cst.tile([C, 8], F32, tag="denom")
            numer = cst.tile([C, 8], F32, tag="numer")

            def conv1_k(t):
                ps1 = conv_tile(y_pad3, w_r["wk1"], 8 * t, 8, H)
                nc.scalar.activation(t1k3[:, 8 * t + 1:8 * t + 9, 1:H + 1],
                                     ps1[:].rearrange("p (r c) -> p r c", c=H),
                                     AF.Relu, bias=b_sb["bk1"][:])

            def conv2_k(t):
                ps2 = conv_tile(t1k3, w_r["wk2"], 8 * t, 8, H)
                kimg = kip.tile([C, 512], F32, tag="kimg")
                nc.scalar.activation(kimg[:], ps2[:], AF.Identity, bias=b_sb["bk2"][:])
                pt = pst.tile([C, 512], F32, tag="tps")
                for i in range(4):
                    nc.tensor.transpose(pt[:, ts(i, C)], kimg[:, ts(i, C)], ident[:])
                nc.vector.tensor_copy(kT[:, ts(t, 512)], pt[:])

            conv1_k(0)
            conv1_k(1)
            for c in range(4):
                if 2 * c + 2 < 8:
                    conv1_k(2 * c + 2)
                if 2 * c + 3 < 8:
                    conv1_k(2 * c + 3)
                conv2_k(2 * c)
                conv2_k(2 * c + 1)

                negmax_c = accp.tile([C, 8], F32, tag="negmax_c")
                d_c = accp.tile([C, 8], F32, tag="d_c")
                n_c = accp.tile([C, 8], F32, tag="n_c")
                for m in range(8):
                    psl = psa.tile([C, 1024], F32, tag="psl")
                    for u in range(2):
                        nc.tensor.matmul(psl[:, ts(u, 512)], qT[:, ts(m, C)],
                                         kT[:, 1024 * c + 512 * u: 1024 * c + 512 * (u + 1)],
                                         start=True, stop=True)
                    # shift: minus the max over one t-block (every-8th key)
                    nc.vector.tensor_reduce(out=negmax_c[:, m:m + 1], in_=psl[:, 0:C],
                                            axis=mybir.AxisListType.X, op=ALU.max,
                                            negate=True)
                    P = ppool.tile([C, 1024], F32, tag="P")
                    nc.scalar.activation(P[:], psl[:], AF.Exp, bias=negmax_c[:, m:m + 1],
                                         scale=1.0, accum_out=d_c[:, m:m + 1])
                    scrap = scrp.tile([C, 1024], BF16, tag="scrap")
                    nc.vector.scalar_tensor_tensor(out=scrap[:], in0=P[:], scalar=1.0,
                                                   in1=v_rep[:, ts(c, 1024)],
                                                   op0=ALU.bypass, op1=ALU.mult,
                                                   accum_out=n_c[:, m:m + 1])
                if c == 0:
                    nc.vector.tensor_copy(negM[:], negmax_c[:])
                    nc.vector.tensor_copy(denom[:], d_c[:])
                    nc.vector.tensor_copy(numer[:], n_c[:])
                else:
                    nmin = scrp.tile([C, 8], F32, tag="nmin")
                    diffs = scrp.tile([C, 16], F32, tag="diffs")
                    alphas = scrp.tile([C, 16], F32, tag="alphas")
                    t8 = scrp.tile([C, 8], F32, tag="t8")
                    nc.vector.tensor_tensor(out=nmin[:], in0=negM[:], in1=negmax_c[:], op=ALU.min)
                    nc.vector.tensor_tensor(out=diffs[:, 0:8], in0=nmin[:], in1=negM[:], op=ALU.subtract)
                    nc.vector.tensor_tensor(out=diffs[:, 8:16], in0=nmin[:], in1=negmax_c[:], op=ALU.subtract)
                    nc.scalar.activation(alphas[:], diffs[:], AF.Exp, scale=1.0)
                    nc.vector.tensor_tensor(out=denom[:], in0=denom[:], in1=alphas[:, 0:8], op=ALU.mult)
                    nc.vector.tensor_tensor(out=t8[:], in0=d_c[:], in1=alphas[:, 8:16], op=ALU.mult)
                    nc.vector.tensor_tensor(out=denom[:], in0=denom[:], in1=t8[:], op=ALU.add)
                    nc.vector.tensor_tensor(out=numer[:], in0=numer[:], in1=alphas[:, 0:8], op=ALU.mult)
                    nc.vector.tensor_tensor(out=t8[:], in0=n_c[:], in1=alphas[:, 8:16], op=ALU.mult)
                    nc.vector.tensor_tensor(out=numer[:], in0=numer[:], in1=t8[:], op=ALU.add)
                    nc.vector.tensor_copy(negM[:], nmin[:])

            recip = cst.tile([C, 8], F32, tag="recip")
            res = cst.tile([C, 8], F32, tag="res")
            nc.vector.reciprocal(recip[:], denom[:])
            nc.vector.tensor_tensor(out=res[:], in0=numer[:], in1=recip[:], op=ALU.mult)
            nc.sync.dma_start(out=dout.ap().rearrange("(co m) one -> co (m one)", m=8),
                              in_=res[:])

    nc.compile()
    return nc


_NC_CACHE = []


def kernel(x, y, z, wq1, bq1, wq2, bq2, wk1, bk1, wk2, bk2):
    x = np.asarray(x, dtype=np.float32)
    y = np.asarray(y, dtype=np.float32)
    z = np.asarray(z, dtype=np.float32)
    B = x.shape[0]
    assert B == N_CORES

    if not _NC_CACHE:
        _NC_CACHE.append(_build_program())
    nc = _NC_CACHE[0]

    # weights: (co, ci, dy, dx) -> (ci, tap*128+co)
    wmap = {}
    for name, w in (("wq1", wq1), ("wq2", wq2), ("wk1", wk1), ("wk2", wk2)):
        wmap[name] = np.ascontiguousarray(
            np.asarray(w, dtype=np.float32).transpose(1, 2, 3, 0).reshape(C, 9 * C))
    bmap = {"bq1": bq1, "bq2": bq2, "bk1": bk1, "bk2": bk2}
    bmap = {n: np.ascontiguousarray(np.asarray(b, dtype=np.float32).reshape(C, 1))
            for n, b in bmap.items()}
    pex = np.ascontiguousarray(_make_pe(C, SQ).reshape(C, SQ))
    pey = np.ascontiguousarray(_make_pe(C, SK).reshape(C, SK))
    ident = np.eye(C, dtype=np.float32)
    # v in t-major key order: store[t*128+co] = z_flat[co*32+t]
    zperm = np.ascontiguousarray(
        z.reshape(B, SK).reshape(B, C, SK // C).transpose(0, 2, 1).reshape(B, 1, SK))

    in_maps = []
    for b in range(B):
        m = {
            "x": np.ascontiguousarray(x[b].reshape(C, SQ)),
            "y": np.ascontiguousarray(y[b].reshape(C, SK)),
            "vz": zperm[b],
            "pex": pex, "pey": pey, "ident": ident,
        }
        m.update(wmap)
        m.update(bmap)
        in_maps.append(m)

    res = run_bass_kernel_spmd(nc, in_maps, core_ids=list(range(N_CORES)))
    out = np.stack([res.results[b]["out"].reshape(SQ, 1) for b in range(B)])
    return out.astype(np.float32)
